# revision 16
# baseline (speedup 1.0000x reference)
"""Trainium2 Bass kernel for a GPT-style transformer block.

Reference computation (B=2, T=2048, D=1024, H=16 heads, causal):
    x = x + attn(LN1(x)) @ Wproj ;  x = x + relu(LN2(x) @ Wfc) @ Wmlp

Distribution over 8 NeuronCores:
  - Head-parallel front: every core computes QKV for its 2 heads over all
    4096 tokens. LN1 is folded into the QKV matmul as a rank-1 correction
    (h1 = (x-mu)*rstd*w + b  =>  W'^T x^T with per-token scale + colsum
    correction), so raw x^T streams straight into the PE.
  - Causal attention per (batch, q-block) in transposed score layout
    S^T[k, q]; both heads' score matmuls are row-tiled into the PE array
    concurrently (contraction rows 0-63 / 64-127); softmax denominators
    fall out of an ones-augmented V column; no max-subtraction (scores
    are provably small).  Score->exp->AV is software-pipelined so the PE
    never stalls on the ScalarE exp.
  - Two mesh AllToAlls (one per batch) re-shard y^T from head-parallel to
    token-parallel; the batch-0 exchange overlaps batch-1 attention.
    Each core owns 256 tokens of each batch.
  - Token-parallel tail: attn-proj + residual + LN2 + MLP for this
    core's 512 tokens.  All big matmuls run as float32r (full PE rate,
    inputs pre-rounded to the fp32r grid on the host).

kernel(**inputs) takes the full unsharded inputs and returns the full
[2, 2048, 1024] output.
"""

import ml_dtypes
import numpy as np

import concourse.bacc as bacc
import concourse.tile as tile
import concourse.mybir as mybir
from concourse.bass_utils import run_bass_kernel_spmd

# Problem shape (hardcoded per the grading contract).
B, T, D = 2, 2048, 1024
H = 16
NC = 8                           # cores
TOK = B * T                      # 4096 tokens
BLK = TOK // NC                  # 512 tokens per core / per block
NB = TOK // BLK                  # 8 token blocks
DC = D // 128                    # 8 d-chunks
FC = 4 * D                       # 4096 mlp hidden
NFC = FC // 128                  # 32 fc chunks
HB = T // NC                     # 256 tokens per core per batch
EPS = 1e-5

F32 = mybir.dt.float32
F32R = mybir.dt.float32r
BF16 = mybir.dt.bfloat16
AL = mybir.AluOpType
AF = mybir.ActivationFunctionType


def v(ap):
    """Bitcast a float32r AP to plain fp32 for DVE/ACT reads."""
    return ap.bitcast(F32)


def round_fp32r(a):
    """Round-to-nearest-even at 12 mantissa bits (the PE fp32r grid)."""
    b = np.ascontiguousarray(a, np.float32).view(np.uint32)
    lsb = (b >> 12) & 1
    return ((b + 0x7FF + lsb) & np.uint32(0xFFFFF000)).view(np.float32)


def build():
    nc = bacc.Bacc("TRN2", target_bir_lowering=False, debug=False,
                   num_devices=NC)

    # ---- DRAM I/O (per-core shards prepared on the host) ----
    io = {}

    def din(name, shape, dt=F32):
        io[name] = nc.dram_tensor(name, shape, dt,
                                  kind="ExternalInput").ap()

    din("xT", [D, TOK], BF16)
    din("x_own", [BLK, D])
    din("wq", [D, 128], F32R)
    din("wk", [D, 128], F32R)
    din("wv", [D, 128], F32R)
    din("bqkv", [128, 3])
    din("ln1w8", [128, DC])
    din("ln1b8", [128, DC], F32R)
    din("ln2w8", [128, DC])
    din("ln2b8", [128, DC])
    din("wproj", [D, D], F32R)
    din("bproj", [1, D])
    din("wfc", [D, FC], BF16)
    din("bfc32", [128, NFC])
    din("wmlp", [FC, D], BF16)
    din("bmlp", [1, D])
    din("masks", [128, 4 * 512], BF16)
    din("ident", [128, 128])
    din("ident_bf", [128, 128], BF16)
    din("ones_d", [128, 33], F32R)
    din("ones_bf", [128, 33], BF16)
    io["out"] = nc.dram_tensor("out", [BLK, D], F32,
                               kind="ExternalOutput").ap()

    with tile.TileContext(nc) as tc:
        _emit(nc, tc, io)
    nc.compile()
    return nc


def _emit(nc, tc, io):
    cst = tc.alloc_tile_pool(name="cst", bufs=1)
    dram = tc.alloc_tile_pool(name="dram", bufs=1, space="DRAM")

    # ---- constants ----
    ident = cst.tile([128, 128], F32, tag="ident")
    nc.sync.dma_start(ident[:], io["ident"][:])
    ident_bf = cst.tile([128, 128], BF16, tag="ident_bf")
    nc.sync.dma_start(ident_bf[:], io["ident_bf"][:])
    ln1w = cst.tile([128, DC], F32, tag="ln1w")
    nc.sync.dma_start(ln1w[:], io["ln1w8"][:])
    ln1b = cst.tile([128, DC], F32R, tag="ln1b")
    nc.sync.dma_start(ln1b[:], io["ln1b8"][:])
    ln2w = cst.tile([128, DC], F32, tag="ln2w")
    nc.sync.dma_start(ln2w[:], io["ln2w8"][:])
    ln2b = cst.tile([128, DC], F32, tag="ln2b")
    nc.sync.dma_start(ln2b[:], io["ln2b8"][:])
    bqkv_t = cst.tile([128, 3], F32, tag="bqkv")
    nc.sync.dma_start(bqkv_t[:], io["bqkv"][:])
    bfc = cst.tile([128, NFC], F32, tag="bfc")
    nc.sync.dma_start(bfc[:], io["bfc32"][:])
    ones = cst.tile([128, 1], F32R, tag="ones")
    nc.sync.dma_start(ones[:], io["ones_d"][:, 0:1])
    ones_bf = cst.tile([128, 1], BF16, tag="ones_bf")
    nc.sync.dma_start(ones_bf[:], io["ones_bf"][:, 0:1])
    eps_t = cst.tile([1, 1], F32, tag="eps")
    nc.vector.memset(eps_t[:], EPS)

    bproj_row = cst.tile([1, D], F32, tag="bprow")
    nc.sync.dma_start(bproj_row[:], io["bproj"][:])
    bproj_bc = cst.tile([128, D], F32, tag="bpbc")
    nc.gpsimd.partition_broadcast(bproj_bc[:], bproj_row[:])
    bmlp_row = cst.tile([1, D], F32, tag="bmrow")
    nc.sync.dma_start(bmlp_row[:], io["bmlp"][:])
    bmlp_bc = cst.tile([128, D], F32, tag="bmbc")
    nc.gpsimd.partition_broadcast(bmlp_bc[:], bmlp_row[:])

    # A2A staging buffers, one exchange per batch ([8 ranks x 128 feats])
    a2a_in = [dram.tile([NC * 128, HB], F32R, tag=f"a2a_in{b}",
                        name=f"a2a_in{b}") for b in range(B)]
    a2a_out = [dram.tile([NC * 128, HB], F32R, tag=f"a2a_out{b}",
                         name=f"a2a_out{b}") for b in range(B)]

    # weight streaming pools — allocated first so their DMAs prefetch
    # under earlier phases (released at the very end, LIFO under big)
    wpp = tc.alloc_tile_pool(name="wpp", bufs=8)
    wfcp = tc.alloc_tile_pool(name="wfcp", bufs=12)
    wmp = tc.alloc_tile_pool(name="wmp", bufs=3)
    # pool big: dies after attention (masks, qt, kt, v_nat)
    big = tc.alloc_tile_pool(name="big", bufs=1)
    # pool pA: dies after phase 1b (vt, scaled weights, x^T stream)
    pA = tc.alloc_tile_pool(name="pA", bufs=1)
    masks = big.tile([128, 4 * 512], BF16, tag="masks")
    nc.sync.dma_start(masks[:], io["masks"][:])
    qt = big.tile([128, TOK], BF16, tag="qt")
    kt = big.tile([128, TOK], BF16, tag="kt")
    vt = pA.tile([128, TOK], BF16, tag="vt")

    # =================== Phase 0: weight prep ===================
    # raw W slices -> wb = W^T ln1_b + b ;  W' = diag(ln1_w) W ;
    # ncolsum = -(W'^T 1)
    ws, ncolsum, wbias = {}, {}, {}
    with tc.tile_pool(name="ph0", bufs=1) as ph0, \
            tc.tile_pool(name="ps0", bufs=2, space="PSUM") as ps0:
        for idx, nm in enumerate(("q", "k", "v")):
            wdram = io["w" + nm]
            raw = ph0.tile([128, D], F32R, tag="raw")
            for c in range(DC):
                nc.sync.dma_start(raw[:, 128 * c:128 * (c + 1)],
                                  wdram[128 * c:128 * (c + 1), :])
            wb_ps = ps0.tile([128, 1], F32, tag="vec")
            for c in range(DC):
                nc.tensor.matmul(wb_ps[:], v(raw[:, 128 * c:128 * (c + 1)]),
                                 v(ln1b[:, c:c + 1]),
                                 start=(c == 0), stop=(c == DC - 1))
            wb = cst.tile([128, 1], F32, tag=f"wb_{nm}")
            nc.vector.tensor_add(wb[:], wb_ps[:], bqkv_t[:, idx:idx + 1])
            wbias[nm] = wb
            wsc = pA.tile([128, D], BF16, tag=f"ws_{nm}", name=f"ws_{nm}")
            for c in range(DC):
                nc.vector.tensor_scalar_mul(
                    wsc[:, 128 * c:128 * (c + 1)],
                    v(raw[:, 128 * c:128 * (c + 1)]), ln1w[:, c:c + 1])
            ws[nm] = wsc
            cs_ps = ps0.tile([128, 1], F32, tag="vec")
            for c in range(DC):
                nc.tensor.matmul(cs_ps[:], wsc[:, 128 * c:128 * (c + 1)],
                                 ones_bf[:],
                                 start=(c == 0), stop=(c == DC - 1))
            ncs = cst.tile([128, 1], F32, tag=f"ncs_{nm}")
            nc.vector.tensor_scalar_mul(ncs[:], cs_ps[:], -1.0)
            ncolsum[nm] = ncs

    # =================== Phase 1: LN1-folded QKV ===================
    # Per 512-token block: stats over x^T, then Q^T/K^T/V^T with the
    # rank-1 LN correction:  out = s*(W'^T x^T) + ncolsum*(s*mu) + wb
    with tc.tile_pool(name="ph1", bufs=2) as ph1, \
            tc.tile_pool(name="xtp", bufs=16) as xtp, \
            tc.tile_pool(name="ps1", bufs=2, space="PSUM") as ps1, \
            tc.tile_pool(name="ps1q", bufs=3, space="PSUM") as ps1q:
        for j in range(NB):
            t0 = BLK * j
            xts = []
            for c in range(DC):
                # cols 0:512 = x^T chunk, 512:1024 = its square
                xt_c = xtp.tile([128, 2 * BLK], BF16, tag="xt")
                nc.sync.dma_start(xt_c[:, 0:BLK],
                                  io["xT"][128 * c:128 * (c + 1),
                                           t0:t0 + BLK])
                nc.scalar.square(xt_c[:, BLK:2 * BLK], xt_c[:, 0:BLK])
                xts.append(xt_c)
            mu_ps = ps1.tile([1, BLK], F32, tag="mu")
            sq_ps = ps1.tile([1, BLK], F32, tag="sq")
            for c in range(DC):
                nc.tensor.matmul(mu_ps[:], ones_bf[:], xts[c][:, 0:BLK],
                                 start=(c == 0), stop=(c == DC - 1))
            for c in range(DC):
                nc.tensor.matmul(sq_ps[:], ones_bf[:],
                                 xts[c][:, BLK:2 * BLK],
                                 start=(c == 0), stop=(c == DC - 1))
            mu_row = ph1.tile([1, BLK], F32, tag="mu_row")
            nc.vector.tensor_scalar_mul(mu_row[:], mu_ps[:], 1.0 / D)
            var_row = ph1.tile([1, BLK], F32, tag="var_row")
            nc.vector.tensor_mul(var_row[:], mu_row[:], mu_row[:])
            nc.vector.scalar_tensor_tensor(
                var_row[:], sq_ps[:], 1.0 / D, var_row[:],
                op0=AL.mult, op1=AL.subtract)
            sd_row = ph1.tile([1, BLK], F32, tag="sd_row")
            nc.scalar.activation(sd_row[:], var_row[:], AF.Sqrt,
                                 bias=eps_t[:])
            s_row = ph1.tile([1, BLK], F32, tag="s_row")
            nc.vector.reciprocal(s_row[:], sd_row[:])
            smu_row = ph1.tile([1, BLK], F32, tag="smu_row")
            nc.vector.tensor_mul(smu_row[:], s_row[:], mu_row[:])
            s_bc = ph1.tile([128, BLK], F32, tag="s_bc")
            nc.gpsimd.partition_broadcast(s_bc[:], s_row[:])
            smu_bc = ph1.tile([128, BLK], F32, tag="smu_bc")
            nc.gpsimd.partition_broadcast(smu_bc[:], smu_row[:])

            for nm, dst in (("q", qt), ("k", kt), ("v", vt)):
                o_ps = ps1q.tile([128, BLK], F32, tag="qkv")
                for c in range(DC):
                    nc.tensor.matmul(o_ps[:],
                                     ws[nm][:, 128 * c:128 * (c + 1)],
                                     xts[c][:, 0:BLK],
                                     start=(c == 0), stop=(c == DC - 1))
                o = dst[:, t0:t0 + BLK]
                nc.vector.tensor_mul(o, o_ps[:], s_bc[:])
                nc.vector.scalar_tensor_tensor(
                    o, smu_bc[:], ncolsum[nm][:], o,
                    op0=AL.mult, op1=AL.add)
                nc.vector.tensor_scalar_add(o, o, wbias[nm][:])

    # =================== Phase 1b: V^T -> V_nat (ones-augmented) =======
    # v_nat[128, 32*130]: token-chunk tc -> cols [130*tc, 130*tc+130):
    #   [0:64) head0 V, [64] ones, [65:129) head1 V, [129] ones
    v_nat = big.tile([128, 32 * 130], BF16, tag="vnat")
    vn3 = v_nat[:].rearrange("p (t s) -> p t s", s=130)
    on3 = io["ones_bf"].rearrange("p (t o) -> p t o", o=1)
    nc.sync.dma_start(vn3[:, :, 64:65], on3[:, 0:32, :])
    nc.sync.dma_start(vn3[:, :, 129:130], on3[:, 0:32, :])
    with tc.tile_pool(name="pstp", bufs=2, space="PSUM") as pstp:
        for tci in range(32):
            tp = pstp.tile([128, 128], BF16, tag="tp")
            nc.tensor.transpose(tp[:], vt[:, 128 * tci:128 * (tci + 1)],
                                ident_bf[:])
            nc.scalar.copy(v_nat[:, 130 * tci:130 * tci + 64], tp[:, 0:64])
            nc.scalar.copy(v_nat[:, 130 * tci + 65:130 * tci + 129],
                           tp[:, 64:128])
    pA.release()

    # =================== Phase 2: attention (pipelined) ===============
    # Work items (b, jb, c): k-chunk c of q-block jb of batch b, both
    # heads at once (score matmuls row-tiled at array rows 0-63/64-127).
    # Scores run LOOKAHEAD items ahead of the AV accumulations so the PE
    # does not stall on ScalarE's exp.
    LOOKAHEAD = 2
    items = []
    for b in range(B):
        for jb in range(4):
            for c in range(4 * jb + 4):
                items.append((b, jb, c))

    with tc.tile_pool(name="att", bufs=6) as att, \
            tc.tile_pool(name="att2", bufs=2) as att2, \
            tc.tile_pool(name="psS", bufs=2, space="PSUM") as psS, \
            tc.tile_pool(name="psA", bufs=2, space="PSUM") as psA:
        av_cur = {}
        pend = []

        def start_scores(it):
            b, jb, c = it
            q0 = 2048 * b + 512 * jb
            k0 = 2048 * b + 128 * c
            es = []
            for h in range(2):
                hr0 = 64 * h
                s_ps = psS.tile([128, BLK], F32, tag=f"s{h}",
                                name=f"s{h}")
                nc.tensor.matmul(s_ps[:],
                                 kt[hr0:hr0 + 64, k0:k0 + 128],
                                 qt[hr0:hr0 + 64, q0:q0 + BLK],
                                 start=True, stop=True)
                e = att.tile([128, BLK], BF16, tag=f"e{h}", name=f"e{h}")
                dc_ = c - 4 * jb
                if dc_ < 0:
                    nc.scalar.activation(e[:], s_ps[:], AF.Exp, scale=0.125)
                else:
                    # diagonal tile: cols < 128*dc_ are fully masked —
                    # zero them and exp/mask only the live region
                    off = 128 * dc_
                    if off:
                        nc.vector.memset(e[:, 0:off], 0.0)
                    nc.scalar.activation(e[:, off:BLK], s_ps[:, off:BLK],
                                         AF.Exp, scale=0.125)
                    nc.vector.tensor_mul(
                        e[:, off:BLK], e[:, off:BLK],
                        masks[:, 512 * dc_ + off:512 * (dc_ + 1)])
                es.append(e)
            return es

        def flush_one():
            (b, jb, c), es = pend.pop(0)
            nk = 4 * jb + 4
            if c == 0:
                av_cur[(b, jb)] = [
                    psA.tile([65, BLK], F32, tag=f"av{h}", name=f"av{h}")
                    for h in range(2)]
            tcg = 16 * b + c
            for h in range(2):
                nc.tensor.matmul(
                    av_cur[(b, jb)][h][:],
                    v_nat[:, 130 * tcg + 65 * h:130 * tcg + 65 * h + 65],
                    es[h][:],
                    start=(c == 0), stop=(c == nk - 1))
            if c == nk - 1:
                avs = av_cur.pop((b, jb))
                for h in range(2):
                    av = avs[h]
                    rrow = att2.tile([1, BLK], F32, tag="rrow")
                    nc.vector.reciprocal(rrow[:], av[64:65, :])
                    rbc = att2.tile([64, BLK], F32, tag="rbc")
                    nc.gpsimd.partition_broadcast(rbc[:], rrow[:])
                    y = att2.tile([64, BLK], F32R, tag="y")
                    nc.vector.tensor_mul(y[:], av[0:64, :], rbc[:])
                    # rank r owns tokens [256r, 256r+256) of each batch;
                    # q-block jb covers ranks 2jb and 2jb+1
                    for half in range(2):
                        rank = 2 * jb + half
                        row0 = 128 * rank + 64 * h
                        nc.sync.dma_start(
                            a2a_in[b][row0:row0 + 64, :],
                            y[:, 256 * half:256 * (half + 1)])

        def fire_a2a(b):
            nc.gpsimd.collective_compute(
                "AllToAll", AL.bypass,
                replica_groups=[list(range(NC))],
                ins=[a2a_in[b].opt()], outs=[a2a_out[b].opt()])

        n_b0 = sum(1 for it in items if it[0] == 0)
        flushed = 0
        for it in items:
            pend.append((it, start_scores(it)))
            if len(pend) > LOOKAHEAD:
                flush_one()
                flushed += 1
                if flushed == n_b0:
                    fire_a2a(0)
        while pend:
            flush_one()
            flushed += 1
            if flushed == n_b0:
                fire_a2a(0)
        fire_a2a(1)
    big.release()

    # =================== Phase 4: attn-proj + residual + LN2 ==========
    ph4 = tc.alloc_tile_pool(name="ph4", bufs=1)
    ph4tmp = tc.alloc_tile_pool(name="ph4tmp", bufs=1)
    yall = []
    for cc in range(DC):
        yc = ph4tmp.tile([128, BLK], F32R, tag=f"yall{cc}",
                         name=f"yall{cc}")
        for b in range(B):
            nc.sync.dma_start(yc[:, 256 * b:256 * (b + 1)],
                              a2a_out[b][128 * cc:128 * (cc + 1), :])
        yall.append(yc)
    xo = []
    for m in range(4):
        xm = ph4tmp.tile([128, D], F32, tag=f"xo{m}", name=f"xo{m}")
        nc.sync.dma_start(xm[:], io["x_own"][128 * m:128 * (m + 1), :])
        xo.append(xm)

    # x2c combo tiles: cols 0:512 = x2^T d-chunk (bf16), 512:1024 = square
    x2c = []
    h2t = []
    for c in range(DC):
        x2c.append(ph4tmp.tile([128, 2 * BLK], BF16, tag=f"x2c{c}",
                               name=f"x2c{c}"))
        h2t.append(ph4.tile([128, BLK], BF16, tag=f"h2t{c}",
                            name=f"h2t{c}"))
    x2 = []
    with tc.tile_pool(name="ps4", bufs=1, space="PSUM") as ps4, \
            tc.tile_pool(name="ph4b", bufs=2) as ph4b:
        wp_sb = []
        for c in range(DC):
            wpc = wpp.tile([128, D], F32R, tag="wp")
            nc.sync.dma_start(wpc[:], io["wproj"][128 * c:128 * (c + 1), :])
            wp_sb.append(wpc)
        zps = [ps4.tile([128, 512], F32, tag=f"zp{i}", name=f"zp{i}")
               for i in range(4)]
        tps = [ps4.tile([128, 128], F32, tag=f"tp2_{i}", name=f"tp2_{i}")
               for i in range(2)]
        for m in range(4):
            for n in range(2):
                zp = zps[2 * (m % 2) + n]
                for c in range(DC):
                    nc.tensor.matmul(
                        zp[:],
                        yall[c][:, 128 * m:128 * (m + 1)],
                        wp_sb[c][:, 512 * n:512 * (n + 1)],
                        start=(c == 0), stop=(c == DC - 1))
            x2m = ph4.tile([128, D], F32, tag=f"x2_{m}", name=f"x2_{m}")
            for n in range(2):
                sl = slice(512 * n, 512 * (n + 1))
                nc.vector.tensor_add(x2m[:, sl], zps[2 * (m % 2) + n][:],
                                     xo[m][:, sl])
                nc.vector.tensor_add(x2m[:, sl], x2m[:, sl],
                                     bproj_bc[:, sl])
            x2.append(x2m)
            for c in range(DC):
                tp = tps[c % 2]
                nc.tensor.transpose(tp[:], x2m[:, 128 * c:128 * (c + 1)],
                                    ident[:])
                nc.scalar.copy(x2c[c][:, 128 * m:128 * (m + 1)], tp[:])
        for c in range(DC):
            nc.scalar.square(x2c[c][:, BLK:2 * BLK], x2c[c][:, 0:BLK])
        mu_ps = ps4.tile([1, BLK], F32, tag="mu2")
        sq_ps = ps4.tile([1, BLK], F32, tag="sq2")
        for c in range(DC):
            nc.tensor.matmul(mu_ps[:], ones_bf[:], x2c[c][:, 0:BLK],
                             start=(c == 0), stop=(c == DC - 1))
        for c in range(DC):
            nc.tensor.matmul(sq_ps[:], ones_bf[:], x2c[c][:, BLK:2 * BLK],
                             start=(c == 0), stop=(c == DC - 1))
        mu_row = ph4b.tile([1, BLK], F32, tag="mu_row2")
        nc.vector.tensor_scalar_mul(mu_row[:], mu_ps[:], 1.0 / D)
        var_row = ph4b.tile([1, BLK], F32, tag="var_row2")
        nc.vector.tensor_mul(var_row[:], mu_row[:], mu_row[:])
        nc.vector.scalar_tensor_tensor(
            var_row[:], sq_ps[:], 1.0 / D, var_row[:],
            op0=AL.mult, op1=AL.subtract)
        sd_row = ph4b.tile([1, BLK], F32, tag="sd_row2")
        nc.scalar.activation(sd_row[:], var_row[:], AF.Sqrt, bias=eps_t[:])
        s_row = ph4b.tile([1, BLK], F32, tag="s_row2")
        nc.vector.reciprocal(s_row[:], sd_row[:])
        s_bc = ph4b.tile([128, BLK], F32, tag="s_bc2")
        nc.gpsimd.partition_broadcast(s_bc[:], s_row[:])
        mu_bc = ph4b.tile([128, BLK], F32, tag="mu_bc2")
        nc.gpsimd.partition_broadcast(mu_bc[:], mu_row[:])
        for c in range(DC):
            nc.vector.tensor_sub(h2t[c][:], x2c[c][:, 0:BLK], mu_bc[:])
            nc.vector.tensor_mul(h2t[c][:], h2t[c][:], s_bc[:])
            nc.vector.tensor_scalar(h2t[c][:], h2t[c][:],
                                    ln2w[:, c:c + 1], ln2b[:, c:c + 1],
                                    op0=AL.mult, op1=AL.add)
    ph4tmp.release()

    # =================== Phase 5: MLP ===================
    rpool = tc.alloc_tile_pool(name="rpool", bufs=1)
    rts = []
    with tc.tile_pool(name="ps5a", bufs=2, space="PSUM") as ps5a:
        for g in range(8):            # fc groups of 4 chunks (512 cols)
            wfc_sb = []
            for c in range(DC):
                wt = wfcp.tile([128, 512], BF16, tag="wfc")
                nc.sync.dma_start(
                    wt[:], io["wfc"][128 * c:128 * (c + 1),
                                     512 * g:512 * (g + 1)])
                wfc_sb.append(wt)
            for fi in range(4):
                fc_i = 4 * g + fi
                gp = ps5a.tile([128, BLK], F32, tag="g")
                for c in range(DC):
                    nc.tensor.matmul(
                        gp[:],
                        wfc_sb[c][:, 128 * fi:128 * (fi + 1)],
                        h2t[c][:],
                        start=(c == 0), stop=(c == DC - 1))
                rt = rpool.tile([128, BLK], BF16, tag=f"r{fc_i}",
                                name=f"r{fc_i}")
                nc.scalar.activation(rt[:], gp[:], AF.Relu,
                                     bias=bfc[:, fc_i:fc_i + 1])
                rts.append(rt)

    with tc.tile_pool(name="ps5b", bufs=1, space="PSUM") as ps5b, \
            tc.tile_pool(name="fin", bufs=2) as fin:
        z2ps = [[None] * 2 for _ in range(4)]
        for m in range(4):
            for n in range(2):
                z2ps[m][n] = ps5b.tile([128, 512], F32, tag=f"z2_{m}_{n}",
                                       name=f"z2_{m}_{n}")
        for fc_i in range(NFC):
            wm = wmp.tile([128, D], BF16, tag="wm")
            nc.sync.dma_start(wm[:],
                              io["wmlp"][128 * fc_i:128 * (fc_i + 1), :])
            for m in range(4):
                for n in range(2):
                    nc.tensor.matmul(
                        z2ps[m][n][:],
                        rts[fc_i][:, 128 * m:128 * (m + 1)],
                        wm[:, 512 * n:512 * (n + 1)],
                        start=(fc_i == 0), stop=(fc_i == NFC - 1))
        for m in range(4):
            fo = fin.tile([128, D], F32, tag="fo")
            for n in range(2):
                sl = slice(512 * n, 512 * (n + 1))
                nc.vector.tensor_add(fo[:, sl], z2ps[m][n][:], x2[m][:, sl])
                nc.vector.tensor_add(fo[:, sl], fo[:, sl], bmlp_bc[:, sl])
            nc.sync.dma_start(io["out"][128 * m:128 * (m + 1), :], fo[:])
    rpool.release()
    ph4.release()
    big2_noop = None
    wmp.release()
    wfcp.release()
    wpp.release()
    dram.release()
    cst.release()


_NC_CACHE = None


def _get_nc():
    global _NC_CACHE
    if _NC_CACHE is None:
        _NC_CACHE = build()
    return _NC_CACHE


def _make_masks():
    kk = np.arange(128)[:, None]
    qq = np.arange(512)[None, :]
    m = np.zeros((128, 4 * 512), np.float32)
    for c in range(4):
        m[:, 512 * c:512 * (c + 1)] = (128 * c + kk <= qq)
    return m


def prepare_in_maps(inputs):
    x = np.asarray(inputs["x"], np.float32)
    w_attn = np.asarray(inputs["w_attn"], np.float32)
    b_attn = np.asarray(inputs["b_attn"], np.float32)
    xf = np.ascontiguousarray(x.reshape(TOK, D))
    xT = np.ascontiguousarray(xf.T.astype(ml_dtypes.bfloat16))
    shared = {
        "xT": xT,
        "ln1w8": np.ascontiguousarray(
            np.asarray(inputs["ln1_w"], np.float32).reshape(DC, 128).T),
        "ln1b8": np.ascontiguousarray(
            np.asarray(inputs["ln1_b"], np.float32).reshape(DC, 128).T),
        "ln2w8": np.ascontiguousarray(
            np.asarray(inputs["ln2_w"], np.float32).reshape(DC, 128).T),
        "ln2b8": np.ascontiguousarray(
            np.asarray(inputs["ln2_b"], np.float32).reshape(DC, 128).T),
        "wproj": round_fp32r(np.asarray(inputs["w_attn_proj"], np.float32)),
        "bproj": np.asarray(
            inputs["b_attn_proj"], np.float32).reshape(1, D),
        "wfc": np.ascontiguousarray(
            np.asarray(inputs["w_fc"], np.float32).astype(
                ml_dtypes.bfloat16)),
        "bfc32": np.ascontiguousarray(
            np.asarray(inputs["b_fc"], np.float32).reshape(NFC, 128).T),
        "wmlp": np.ascontiguousarray(
            np.asarray(inputs["w_mlp_proj"], np.float32).astype(
                ml_dtypes.bfloat16)),
        "bmlp": np.asarray(
            inputs["b_mlp_proj"], np.float32).reshape(1, D),
        "masks": _make_masks().astype(ml_dtypes.bfloat16),
        "ident": np.eye(128, dtype=np.float32),
        "ident_bf": np.eye(128, dtype=np.float32).astype(ml_dtypes.bfloat16),
        "ones_d": np.ones((128, 33), np.float32),
        "ones_bf": np.ones((128, 33), ml_dtypes.bfloat16),
    }
    in_maps = []
    for i in range(NC):
        f0 = 128 * i
        m = dict(shared)
        # rank i owns tokens [256i, 256i+256) of each batch
        m["x_own"] = np.ascontiguousarray(
            np.concatenate([x[0, HB * i:HB * (i + 1)],
                            x[1, HB * i:HB * (i + 1)]], axis=0))
        m["wq"] = round_fp32r(w_attn[:, f0:f0 + 128])
        m["wk"] = round_fp32r(w_attn[:, D + f0:D + f0 + 128])
        m["wv"] = round_fp32r(w_attn[:, 2 * D + f0:2 * D + f0 + 128])
        m["bqkv"] = np.ascontiguousarray(np.stack(
            [b_attn[f0:f0 + 128], b_attn[D + f0:D + f0 + 128],
             b_attn[2 * D + f0:2 * D + f0 + 128]], axis=1))
        in_maps.append(m)
    return in_maps


def run(inputs, trace=False):
    nc = _get_nc()
    in_maps = prepare_in_maps(inputs)
    res = run_bass_kernel_spmd(nc, in_maps, list(range(NC)), trace=trace)
    full = np.empty((B, T, D), np.float32)
    for i in range(NC):
        blk = res.results[i]["out"]
        full[0, HB * i:HB * (i + 1)] = blk[0:HB]
        full[1, HB * i:HB * (i + 1)] = blk[HB:2 * HB]
    return full, res


def kernel(**inputs):
    full, _ = run(inputs, trace=False)
    return full


# revision 17
# speedup vs baseline: 1.0227x; 1.0227x over previous
"""Trainium2 Bass kernel for a GPT-style transformer block.

Reference computation (B=2, T=2048, D=1024, H=16 heads, causal):
    x = x + attn(LN1(x)) @ Wproj ;  x = x + relu(LN2(x) @ Wfc) @ Wmlp

Distribution over 8 NeuronCores:
  - Head-parallel front: every core computes QKV for its 2 heads over all
    4096 tokens. LN1 is folded into the QKV matmul as a rank-1 correction
    (h1 = (x-mu)*rstd*w + b  =>  W'^T x^T with per-token scale + colsum
    correction), so raw x^T streams straight into the PE.
  - Causal attention per (batch, q-block) in transposed score layout
    S^T[k, q]; both heads' score matmuls are row-tiled into the PE array
    concurrently (contraction rows 0-63 / 64-127); softmax denominators
    fall out of an ones-augmented V column; no max-subtraction (scores
    are provably small).  Score->exp->AV is software-pipelined so the PE
    never stalls on the ScalarE exp.
  - Two mesh AllToAlls (one per batch) re-shard y^T from head-parallel to
    token-parallel; the batch-0 exchange overlaps batch-1 attention.
    Each core owns 256 tokens of each batch.
  - Token-parallel tail: attn-proj + residual + LN2 + MLP for this
    core's 512 tokens.  All big matmuls run as float32r (full PE rate,
    inputs pre-rounded to the fp32r grid on the host).

kernel(**inputs) takes the full unsharded inputs and returns the full
[2, 2048, 1024] output.
"""

import ml_dtypes
import numpy as np

import concourse.bacc as bacc
import concourse.tile as tile
import concourse.mybir as mybir
from concourse.bass_utils import run_bass_kernel_spmd

# Problem shape (hardcoded per the grading contract).
B, T, D = 2, 2048, 1024
H = 16
NC = 8                           # cores
TOK = B * T                      # 4096 tokens
BLK = TOK // NC                  # 512 tokens per core / per block
NB = TOK // BLK                  # 8 token blocks
DC = D // 128                    # 8 d-chunks
FC = 4 * D                       # 4096 mlp hidden
NFC = FC // 128                  # 32 fc chunks
HB = T // NC                     # 256 tokens per core per batch
EPS = 1e-5

F32 = mybir.dt.float32
F32R = mybir.dt.float32r
BF16 = mybir.dt.bfloat16
AL = mybir.AluOpType
AF = mybir.ActivationFunctionType


def v(ap):
    """Bitcast a float32r AP to plain fp32 for DVE/ACT reads."""
    return ap.bitcast(F32)


def round_fp32r(a):
    """Round-to-nearest-even at 12 mantissa bits (the PE fp32r grid)."""
    b = np.ascontiguousarray(a, np.float32).view(np.uint32)
    lsb = (b >> 12) & 1
    return ((b + 0x7FF + lsb) & np.uint32(0xFFFFF000)).view(np.float32)


def build():
    nc = bacc.Bacc("TRN2", target_bir_lowering=False, debug=False,
                   num_devices=NC)

    # ---- DRAM I/O (per-core shards prepared on the host) ----
    io = {}

    def din(name, shape, dt=F32):
        io[name] = nc.dram_tensor(name, shape, dt,
                                  kind="ExternalInput").ap()

    din("xT", [D, TOK], BF16)
    din("x_own", [BLK, D])
    din("wq", [D, 128], F32R)
    din("wk", [D, 128], F32R)
    din("wv", [D, 128], F32R)
    din("bqkv", [128, 3])
    din("ln1w8", [128, DC])
    din("ln1b8", [128, DC], F32R)
    din("ln2w8", [128, DC])
    din("ln2b8", [128, DC])
    din("wproj", [D, D], F32R)
    din("bproj", [1, D])
    din("wfc", [D, FC], BF16)
    din("bfc32", [128, NFC])
    din("wmlp", [FC, D], BF16)
    din("bmlp", [1, D])
    din("masks", [128, 4 * 512], BF16)
    din("ident", [128, 128])
    din("ident_bf", [128, 128], BF16)
    din("ones_d", [128, 33], F32R)
    din("ones_bf", [128, 33], BF16)
    io["out"] = nc.dram_tensor("out", [BLK, D], F32,
                               kind="ExternalOutput").ap()

    with tile.TileContext(nc) as tc:
        _emit(nc, tc, io)
    nc.compile()
    return nc


def _emit(nc, tc, io):
    cst = tc.alloc_tile_pool(name="cst", bufs=1)
    dram = tc.alloc_tile_pool(name="dram", bufs=1, space="DRAM")

    # ---- constants ----
    ident = cst.tile([128, 128], F32, tag="ident")
    nc.sync.dma_start(ident[:], io["ident"][:])
    ident_bf = cst.tile([128, 128], BF16, tag="ident_bf")
    nc.sync.dma_start(ident_bf[:], io["ident_bf"][:])
    ln1w = cst.tile([128, DC], F32, tag="ln1w")
    nc.sync.dma_start(ln1w[:], io["ln1w8"][:])
    ln1b = cst.tile([128, DC], F32R, tag="ln1b")
    nc.sync.dma_start(ln1b[:], io["ln1b8"][:])
    ln2w = cst.tile([128, DC], F32, tag="ln2w")
    nc.sync.dma_start(ln2w[:], io["ln2w8"][:])
    ln2b = cst.tile([128, DC], F32, tag="ln2b")
    nc.sync.dma_start(ln2b[:], io["ln2b8"][:])
    bqkv_t = cst.tile([128, 3], F32, tag="bqkv")
    nc.sync.dma_start(bqkv_t[:], io["bqkv"][:])
    bfc = cst.tile([128, NFC], F32, tag="bfc")
    nc.sync.dma_start(bfc[:], io["bfc32"][:])
    ones = cst.tile([128, 1], F32R, tag="ones")
    nc.sync.dma_start(ones[:], io["ones_d"][:, 0:1])
    ones_bf = cst.tile([128, 1], BF16, tag="ones_bf")
    nc.sync.dma_start(ones_bf[:], io["ones_bf"][:, 0:1])
    eps_t = cst.tile([1, 1], F32, tag="eps")
    nc.vector.memset(eps_t[:], EPS)

    bproj_row = cst.tile([1, D], F32, tag="bprow")
    nc.sync.dma_start(bproj_row[:], io["bproj"][:])
    bproj_bc = cst.tile([128, D], F32, tag="bpbc")
    nc.gpsimd.partition_broadcast(bproj_bc[:], bproj_row[:])
    bmlp_row = cst.tile([1, D], F32, tag="bmrow")
    nc.sync.dma_start(bmlp_row[:], io["bmlp"][:])
    bmlp_bc = cst.tile([128, D], F32, tag="bmbc")
    nc.gpsimd.partition_broadcast(bmlp_bc[:], bmlp_row[:])

    # A2A staging buffers, one exchange per batch ([8 ranks x 128 feats])
    a2a_in = [dram.tile([NC * 128, HB], F32R, tag=f"a2a_in{b}",
                        name=f"a2a_in{b}") for b in range(B)]
    a2a_out = [dram.tile([NC * 128, HB], F32R, tag=f"a2a_out{b}",
                         name=f"a2a_out{b}") for b in range(B)]

    # weight streaming pools — allocated first so their DMAs prefetch
    # under earlier phases (released at the very end, LIFO under big)
    wpp = tc.alloc_tile_pool(name="wpp", bufs=8)
    wfcp = tc.alloc_tile_pool(name="wfcp", bufs=12)
    wmp = tc.alloc_tile_pool(name="wmp", bufs=3)
    # pool big: dies after attention (masks, qt, kt, v_nat)
    big = tc.alloc_tile_pool(name="big", bufs=1)
    # pool pA: dies after phase 1b (vt, scaled weights, x^T stream)
    pA = tc.alloc_tile_pool(name="pA", bufs=1)
    masks = big.tile([128, 4 * 512], BF16, tag="masks")
    nc.sync.dma_start(masks[:], io["masks"][:])
    qt = big.tile([128, TOK], BF16, tag="qt")
    kt = big.tile([128, TOK], BF16, tag="kt")
    vt = pA.tile([128, TOK], BF16, tag="vt")

    # =================== Phase 0: weight prep ===================
    # raw W slices -> wb = W^T ln1_b + b ;  W' = diag(ln1_w) W ;
    # ncolsum = -(W'^T 1)
    ws, ncolsum, wbias = {}, {}, {}
    with tc.tile_pool(name="ph0", bufs=1) as ph0, \
            tc.tile_pool(name="ps0", bufs=2, space="PSUM") as ps0:
        for idx, nm in enumerate(("q", "k", "v")):
            wdram = io["w" + nm]
            raw = ph0.tile([128, D], F32R, tag="raw")
            for c in range(DC):
                nc.sync.dma_start(raw[:, 128 * c:128 * (c + 1)],
                                  wdram[128 * c:128 * (c + 1), :])
            wb_ps = ps0.tile([128, 1], F32, tag="vec")
            for c in range(DC):
                nc.tensor.matmul(wb_ps[:], v(raw[:, 128 * c:128 * (c + 1)]),
                                 v(ln1b[:, c:c + 1]),
                                 start=(c == 0), stop=(c == DC - 1))
            wb = cst.tile([128, 1], F32, tag=f"wb_{nm}")
            nc.vector.tensor_add(wb[:], wb_ps[:], bqkv_t[:, idx:idx + 1])
            wbias[nm] = wb
            wsc = pA.tile([128, D], BF16, tag=f"ws_{nm}", name=f"ws_{nm}")
            for c in range(DC):
                nc.vector.tensor_scalar_mul(
                    wsc[:, 128 * c:128 * (c + 1)],
                    v(raw[:, 128 * c:128 * (c + 1)]), ln1w[:, c:c + 1])
            ws[nm] = wsc
            cs_ps = ps0.tile([128, 1], F32, tag="vec")
            for c in range(DC):
                nc.tensor.matmul(cs_ps[:], wsc[:, 128 * c:128 * (c + 1)],
                                 ones_bf[:],
                                 start=(c == 0), stop=(c == DC - 1))
            ncs = cst.tile([128, 1], F32, tag=f"ncs_{nm}")
            nc.vector.tensor_scalar_mul(ncs[:], cs_ps[:], -1.0)
            ncolsum[nm] = ncs

    # =================== Phase 1: LN1-folded QKV ===================
    # Per 512-token block: stats over x^T, then Q^T/K^T/V^T with the
    # rank-1 LN correction:  out = s*(W'^T x^T) + ncolsum*(s*mu) + wb
    with tc.tile_pool(name="ph1", bufs=2) as ph1, \
            tc.tile_pool(name="xtp", bufs=16) as xtp, \
            tc.tile_pool(name="ps1", bufs=2, space="PSUM") as ps1, \
            tc.tile_pool(name="ps1q", bufs=3, space="PSUM") as ps1q:
        for j in range(NB):
            t0 = BLK * j
            xts = []
            for c in range(DC):
                # cols 0:512 = x^T chunk, 512:1024 = its square
                xt_c = xtp.tile([128, 2 * BLK], BF16, tag="xt")
                nc.sync.dma_start(xt_c[:, 0:BLK],
                                  io["xT"][128 * c:128 * (c + 1),
                                           t0:t0 + BLK])
                nc.scalar.square(xt_c[:, BLK:2 * BLK], xt_c[:, 0:BLK])
                xts.append(xt_c)
            mu_ps = ps1.tile([1, BLK], F32, tag="mu")
            sq_ps = ps1.tile([1, BLK], F32, tag="sq")
            for c in range(DC):
                nc.tensor.matmul(mu_ps[:], ones_bf[:], xts[c][:, 0:BLK],
                                 start=(c == 0), stop=(c == DC - 1))
            for c in range(DC):
                nc.tensor.matmul(sq_ps[:], ones_bf[:],
                                 xts[c][:, BLK:2 * BLK],
                                 start=(c == 0), stop=(c == DC - 1))
            mu_row = ph1.tile([1, BLK], F32, tag="mu_row")
            nc.vector.tensor_scalar_mul(mu_row[:], mu_ps[:], 1.0 / D)
            var_row = ph1.tile([1, BLK], F32, tag="var_row")
            nc.vector.tensor_mul(var_row[:], mu_row[:], mu_row[:])
            nc.vector.scalar_tensor_tensor(
                var_row[:], sq_ps[:], 1.0 / D, var_row[:],
                op0=AL.mult, op1=AL.subtract)
            sd_row = ph1.tile([1, BLK], F32, tag="sd_row")
            nc.scalar.activation(sd_row[:], var_row[:], AF.Sqrt,
                                 bias=eps_t[:])
            s_row = ph1.tile([1, BLK], F32, tag="s_row")
            nc.vector.reciprocal(s_row[:], sd_row[:])
            smu_row = ph1.tile([1, BLK], F32, tag="smu_row")
            nc.vector.tensor_mul(smu_row[:], s_row[:], mu_row[:])
            s_bc = ph1.tile([128, BLK], F32, tag="s_bc")
            nc.gpsimd.partition_broadcast(s_bc[:], s_row[:])
            smu_bc = ph1.tile([128, BLK], F32, tag="smu_bc")
            nc.gpsimd.partition_broadcast(smu_bc[:], smu_row[:])

            for nm, dst in (("q", qt), ("k", kt), ("v", vt)):
                o_ps = ps1q.tile([128, BLK], F32, tag="qkv")
                for c in range(DC):
                    nc.tensor.matmul(o_ps[:],
                                     ws[nm][:, 128 * c:128 * (c + 1)],
                                     xts[c][:, 0:BLK],
                                     start=(c == 0), stop=(c == DC - 1))
                o = dst[:, t0:t0 + BLK]
                nc.vector.tensor_mul(o, o_ps[:], s_bc[:])
                nc.vector.scalar_tensor_tensor(
                    o, smu_bc[:], ncolsum[nm][:], o,
                    op0=AL.mult, op1=AL.add)
                nc.vector.tensor_scalar_add(o, o, wbias[nm][:])

    # =================== Phase 1b: V^T -> V_nat (ones-augmented) =======
    # v_nat[128, 32*130]: token-chunk tc -> cols [130*tc, 130*tc+130):
    #   [0:64) head0 V, [64] ones, [65:129) head1 V, [129] ones
    v_nat = big.tile([128, 32 * 130], BF16, tag="vnat")
    vn3 = v_nat[:].rearrange("p (t s) -> p t s", s=130)
    on3 = io["ones_bf"].rearrange("p (t o) -> p t o", o=1)
    nc.sync.dma_start(vn3[:, :, 64:65], on3[:, 0:32, :])
    nc.sync.dma_start(vn3[:, :, 129:130], on3[:, 0:32, :])
    with tc.tile_pool(name="pstp", bufs=2, space="PSUM") as pstp:
        for tci in range(32):
            tp = pstp.tile([128, 128], BF16, tag="tp")
            nc.tensor.transpose(tp[:], vt[:, 128 * tci:128 * (tci + 1)],
                                ident_bf[:])
            nc.scalar.copy(v_nat[:, 130 * tci:130 * tci + 64], tp[:, 0:64])
            nc.scalar.copy(v_nat[:, 130 * tci + 65:130 * tci + 129],
                           tp[:, 64:128])
    pA.release()

    # =================== Phase 2: attention (pipelined) ===============
    # Work items (b, jb, c): k-chunk c of q-block jb of batch b, both
    # heads at once (score matmuls row-tiled at array rows 0-63/64-127).
    # Scores run LOOKAHEAD items ahead of the AV accumulations so the PE
    # does not stall on ScalarE's exp.
    LOOKAHEAD = 2
    items = []
    for b in range(B):
        for jb in range(4):
            for c in range(4 * jb + 4):
                items.append((b, jb, c))

    with tc.tile_pool(name="att", bufs=6) as att, \
            tc.tile_pool(name="att2", bufs=2) as att2, \
            tc.tile_pool(name="psS", bufs=2, space="PSUM") as psS, \
            tc.tile_pool(name="psA", bufs=2, space="PSUM") as psA:
        av_cur = {}
        pend = []

        def start_scores(it):
            b, jb, c = it
            q0 = 2048 * b + 512 * jb
            k0 = 2048 * b + 128 * c
            es = []
            for h in range(2):
                hr0 = 64 * h
                s_ps = psS.tile([128, BLK], F32, tag=f"s{h}",
                                name=f"s{h}")
                nc.tensor.matmul(s_ps[:],
                                 kt[hr0:hr0 + 64, k0:k0 + 128],
                                 qt[hr0:hr0 + 64, q0:q0 + BLK],
                                 start=True, stop=True)
                e = att.tile([128, BLK], BF16, tag=f"e{h}", name=f"e{h}")
                dc_ = c - 4 * jb
                if dc_ < 0:
                    nc.scalar.activation(e[:], s_ps[:], AF.Exp, scale=0.125)
                else:
                    # diagonal tile: cols < 128*dc_ are fully masked —
                    # zero them and exp/mask only the live region
                    off = 128 * dc_
                    if off:
                        nc.vector.memset(e[:, 0:off], 0.0)
                    nc.scalar.activation(e[:, off:BLK], s_ps[:, off:BLK],
                                         AF.Exp, scale=0.125)
                    nc.vector.tensor_mul(
                        e[:, off:BLK], e[:, off:BLK],
                        masks[:, 512 * dc_ + off:512 * (dc_ + 1)])
                es.append(e)
            return es

        def flush_one():
            (b, jb, c), es = pend.pop(0)
            nk = 4 * jb + 4
            if c == 0:
                av_cur[(b, jb)] = [
                    psA.tile([65, BLK], F32, tag=f"av{h}", name=f"av{h}")
                    for h in range(2)]
            tcg = 16 * b + c
            for h in range(2):
                nc.tensor.matmul(
                    av_cur[(b, jb)][h][:],
                    v_nat[:, 130 * tcg + 65 * h:130 * tcg + 65 * h + 65],
                    es[h][:],
                    start=(c == 0), stop=(c == nk - 1))
            if c == nk - 1:
                avs = av_cur.pop((b, jb))
                for h in range(2):
                    av = avs[h]
                    rrow = att2.tile([1, BLK], F32, tag="rrow")
                    nc.vector.reciprocal(rrow[:], av[64:65, :])
                    rbc = att2.tile([64, BLK], F32, tag="rbc")
                    nc.gpsimd.partition_broadcast(rbc[:], rrow[:])
                    y = att2.tile([64, BLK], F32R, tag="y")
                    nc.vector.tensor_mul(y[:], av[0:64, :], rbc[:])
                    # rank r owns tokens [256r, 256r+256) of each batch;
                    # q-block jb covers ranks 2jb and 2jb+1
                    for half in range(2):
                        rank = 2 * jb + half
                        row0 = 128 * rank + 64 * h
                        nc.sync.dma_start(
                            a2a_in[b][row0:row0 + 64, :],
                            y[:, 256 * half:256 * (half + 1)])

        def fire_a2a(b):
            nc.gpsimd.collective_compute(
                "AllToAll", AL.bypass,
                replica_groups=[list(range(NC))],
                ins=[a2a_in[b].opt()], outs=[a2a_out[b].opt()])

        n_b0 = sum(1 for it in items if it[0] == 0)
        flushed = 0
        for it in items:
            pend.append((it, start_scores(it)))
            if len(pend) > LOOKAHEAD:
                flush_one()
                flushed += 1
                if flushed == n_b0:
                    fire_a2a(0)
        while pend:
            flush_one()
            flushed += 1
            if flushed == n_b0:
                fire_a2a(0)
        fire_a2a(1)
    big.release()

    # =================== Phase 4: attn-proj + residual + LN2 ==========
    ph4 = tc.alloc_tile_pool(name="ph4", bufs=1)
    ph4tmp = tc.alloc_tile_pool(name="ph4tmp", bufs=1)
    yall = []
    for cc in range(DC):
        ycs = []
        for b in range(B):
            yc = ph4tmp.tile([128, HB], F32R, tag=f"yall{b}_{cc}",
                             name=f"yall{b}_{cc}")
            nc.sync.dma_start(yc[:],
                              a2a_out[b][128 * cc:128 * (cc + 1), :])
            ycs.append(yc)
        yall.append(ycs)
    xo = []
    for m in range(4):
        xm = ph4tmp.tile([128, D], F32, tag=f"xo{m}", name=f"xo{m}")
        nc.sync.dma_start(xm[:], io["x_own"][128 * m:128 * (m + 1), :])
        xo.append(xm)

    # x2c combo tiles: cols 0:512 = x2^T d-chunk (bf16), 512:1024 = square
    x2c = []
    h2t = []
    for c in range(DC):
        x2c.append(ph4tmp.tile([128, 2 * BLK], BF16, tag=f"x2c{c}",
                               name=f"x2c{c}"))
        h2t.append(ph4.tile([128, BLK], BF16, tag=f"h2t{c}",
                            name=f"h2t{c}"))
    x2 = []
    with tc.tile_pool(name="ps4", bufs=1, space="PSUM") as ps4, \
            tc.tile_pool(name="ph4b", bufs=2) as ph4b:
        wp_sb = []
        for c in range(DC):
            wpc = wpp.tile([128, D], F32R, tag="wp")
            nc.sync.dma_start(wpc[:], io["wproj"][128 * c:128 * (c + 1), :])
            wp_sb.append(wpc)
        zps = [ps4.tile([128, 512], F32, tag=f"zp{i}", name=f"zp{i}")
               for i in range(4)]
        tps = [ps4.tile([128, 128], F32, tag=f"tp2_{i}", name=f"tp2_{i}")
               for i in range(2)]
        for m in range(4):
            mb, ms = divmod(m, 2)
            for n in range(2):
                zp = zps[2 * (m % 2) + n]
                for c in range(DC):
                    nc.tensor.matmul(
                        zp[:],
                        yall[c][mb][:, 128 * ms:128 * (ms + 1)],
                        wp_sb[c][:, 512 * n:512 * (n + 1)],
                        start=(c == 0), stop=(c == DC - 1))
            x2m = ph4.tile([128, D], F32, tag=f"x2_{m}", name=f"x2_{m}")
            for n in range(2):
                sl = slice(512 * n, 512 * (n + 1))
                nc.vector.tensor_add(x2m[:, sl], zps[2 * (m % 2) + n][:],
                                     xo[m][:, sl])
                nc.vector.tensor_add(x2m[:, sl], x2m[:, sl],
                                     bproj_bc[:, sl])
            x2.append(x2m)
            for c in range(DC):
                tp = tps[c % 2]
                nc.tensor.transpose(tp[:], x2m[:, 128 * c:128 * (c + 1)],
                                    ident[:])
                nc.scalar.copy(x2c[c][:, 128 * m:128 * (m + 1)], tp[:])
        for c in range(DC):
            nc.scalar.square(x2c[c][:, BLK:2 * BLK], x2c[c][:, 0:BLK])
        mu_ps = ps4.tile([1, BLK], F32, tag="mu2")
        sq_ps = ps4.tile([1, BLK], F32, tag="sq2")
        for c in range(DC):
            nc.tensor.matmul(mu_ps[:], ones_bf[:], x2c[c][:, 0:BLK],
                             start=(c == 0), stop=(c == DC - 1))
        for c in range(DC):
            nc.tensor.matmul(sq_ps[:], ones_bf[:], x2c[c][:, BLK:2 * BLK],
                             start=(c == 0), stop=(c == DC - 1))
        mu_row = ph4b.tile([1, BLK], F32, tag="mu_row2")
        nc.vector.tensor_scalar_mul(mu_row[:], mu_ps[:], 1.0 / D)
        var_row = ph4b.tile([1, BLK], F32, tag="var_row2")
        nc.vector.tensor_mul(var_row[:], mu_row[:], mu_row[:])
        nc.vector.scalar_tensor_tensor(
            var_row[:], sq_ps[:], 1.0 / D, var_row[:],
            op0=AL.mult, op1=AL.subtract)
        sd_row = ph4b.tile([1, BLK], F32, tag="sd_row2")
        nc.scalar.activation(sd_row[:], var_row[:], AF.Sqrt, bias=eps_t[:])
        s_row = ph4b.tile([1, BLK], F32, tag="s_row2")
        nc.vector.reciprocal(s_row[:], sd_row[:])
        s_bc = ph4b.tile([128, BLK], F32, tag="s_bc2")
        nc.gpsimd.partition_broadcast(s_bc[:], s_row[:])
        mu_bc = ph4b.tile([128, BLK], F32, tag="mu_bc2")
        nc.gpsimd.partition_broadcast(mu_bc[:], mu_row[:])
        for c in range(DC):
            nc.vector.tensor_sub(h2t[c][:], x2c[c][:, 0:BLK], mu_bc[:])
            nc.vector.tensor_mul(h2t[c][:], h2t[c][:], s_bc[:])
            nc.vector.tensor_scalar(h2t[c][:], h2t[c][:],
                                    ln2w[:, c:c + 1], ln2b[:, c:c + 1],
                                    op0=AL.mult, op1=AL.add)
    ph4tmp.release()

    # =================== Phase 5: MLP ===================
    rpool = tc.alloc_tile_pool(name="rpool", bufs=1)
    rts = []
    with tc.tile_pool(name="ps5a", bufs=2, space="PSUM") as ps5a:
        for g in range(8):            # fc groups of 4 chunks (512 cols)
            wfc_sb = []
            for c in range(DC):
                wt = wfcp.tile([128, 512], BF16, tag="wfc")
                nc.sync.dma_start(
                    wt[:], io["wfc"][128 * c:128 * (c + 1),
                                     512 * g:512 * (g + 1)])
                wfc_sb.append(wt)
            for fi in range(4):
                fc_i = 4 * g + fi
                gp = ps5a.tile([128, BLK], F32, tag="g")
                for c in range(DC):
                    nc.tensor.matmul(
                        gp[:],
                        wfc_sb[c][:, 128 * fi:128 * (fi + 1)],
                        h2t[c][:],
                        start=(c == 0), stop=(c == DC - 1))
                rt = rpool.tile([128, BLK], BF16, tag=f"r{fc_i}",
                                name=f"r{fc_i}")
                nc.scalar.activation(rt[:], gp[:], AF.Relu,
                                     bias=bfc[:, fc_i:fc_i + 1])
                rts.append(rt)

    with tc.tile_pool(name="ps5b", bufs=1, space="PSUM") as ps5b, \
            tc.tile_pool(name="fin", bufs=2) as fin:
        z2ps = [[None] * 2 for _ in range(4)]
        for m in range(4):
            for n in range(2):
                z2ps[m][n] = ps5b.tile([128, 512], F32, tag=f"z2_{m}_{n}",
                                       name=f"z2_{m}_{n}")
        for fc_i in range(NFC):
            wm = wmp.tile([128, D], BF16, tag="wm")
            nc.sync.dma_start(wm[:],
                              io["wmlp"][128 * fc_i:128 * (fc_i + 1), :])
            for m in range(4):
                for n in range(2):
                    nc.tensor.matmul(
                        z2ps[m][n][:],
                        rts[fc_i][:, 128 * m:128 * (m + 1)],
                        wm[:, 512 * n:512 * (n + 1)],
                        start=(fc_i == 0), stop=(fc_i == NFC - 1))
        for m in range(4):
            fo = fin.tile([128, D], F32, tag="fo")
            for n in range(2):
                sl = slice(512 * n, 512 * (n + 1))
                nc.vector.tensor_add(fo[:, sl], z2ps[m][n][:], x2[m][:, sl])
                nc.vector.tensor_add(fo[:, sl], fo[:, sl], bmlp_bc[:, sl])
            nc.sync.dma_start(io["out"][128 * m:128 * (m + 1), :], fo[:])
    rpool.release()
    ph4.release()
    big2_noop = None
    wmp.release()
    wfcp.release()
    wpp.release()
    dram.release()
    cst.release()


_NC_CACHE = None


def _get_nc():
    global _NC_CACHE
    if _NC_CACHE is None:
        _NC_CACHE = build()
    return _NC_CACHE


def _make_masks():
    kk = np.arange(128)[:, None]
    qq = np.arange(512)[None, :]
    m = np.zeros((128, 4 * 512), np.float32)
    for c in range(4):
        m[:, 512 * c:512 * (c + 1)] = (128 * c + kk <= qq)
    return m


def prepare_in_maps(inputs):
    x = np.asarray(inputs["x"], np.float32)
    w_attn = np.asarray(inputs["w_attn"], np.float32)
    b_attn = np.asarray(inputs["b_attn"], np.float32)
    xf = np.ascontiguousarray(x.reshape(TOK, D))
    xT = np.ascontiguousarray(xf.T.astype(ml_dtypes.bfloat16))
    shared = {
        "xT": xT,
        "ln1w8": np.ascontiguousarray(
            np.asarray(inputs["ln1_w"], np.float32).reshape(DC, 128).T),
        "ln1b8": np.ascontiguousarray(
            np.asarray(inputs["ln1_b"], np.float32).reshape(DC, 128).T),
        "ln2w8": np.ascontiguousarray(
            np.asarray(inputs["ln2_w"], np.float32).reshape(DC, 128).T),
        "ln2b8": np.ascontiguousarray(
            np.asarray(inputs["ln2_b"], np.float32).reshape(DC, 128).T),
        "wproj": round_fp32r(np.asarray(inputs["w_attn_proj"], np.float32)),
        "bproj": np.asarray(
            inputs["b_attn_proj"], np.float32).reshape(1, D),
        "wfc": np.ascontiguousarray(
            np.asarray(inputs["w_fc"], np.float32).astype(
                ml_dtypes.bfloat16)),
        "bfc32": np.ascontiguousarray(
            np.asarray(inputs["b_fc"], np.float32).reshape(NFC, 128).T),
        "wmlp": np.ascontiguousarray(
            np.asarray(inputs["w_mlp_proj"], np.float32).astype(
                ml_dtypes.bfloat16)),
        "bmlp": np.asarray(
            inputs["b_mlp_proj"], np.float32).reshape(1, D),
        "masks": _make_masks().astype(ml_dtypes.bfloat16),
        "ident": np.eye(128, dtype=np.float32),
        "ident_bf": np.eye(128, dtype=np.float32).astype(ml_dtypes.bfloat16),
        "ones_d": np.ones((128, 33), np.float32),
        "ones_bf": np.ones((128, 33), ml_dtypes.bfloat16),
    }
    in_maps = []
    for i in range(NC):
        f0 = 128 * i
        m = dict(shared)
        # rank i owns tokens [256i, 256i+256) of each batch
        m["x_own"] = np.ascontiguousarray(
            np.concatenate([x[0, HB * i:HB * (i + 1)],
                            x[1, HB * i:HB * (i + 1)]], axis=0))
        m["wq"] = round_fp32r(w_attn[:, f0:f0 + 128])
        m["wk"] = round_fp32r(w_attn[:, D + f0:D + f0 + 128])
        m["wv"] = round_fp32r(w_attn[:, 2 * D + f0:2 * D + f0 + 128])
        m["bqkv"] = np.ascontiguousarray(np.stack(
            [b_attn[f0:f0 + 128], b_attn[D + f0:D + f0 + 128],
             b_attn[2 * D + f0:2 * D + f0 + 128]], axis=1))
        in_maps.append(m)
    return in_maps


def run(inputs, trace=False):
    nc = _get_nc()
    in_maps = prepare_in_maps(inputs)
    res = run_bass_kernel_spmd(nc, in_maps, list(range(NC)), trace=trace)
    full = np.empty((B, T, D), np.float32)
    for i in range(NC):
        blk = res.results[i]["out"]
        full[0, HB * i:HB * (i + 1)] = blk[0:HB]
        full[1, HB * i:HB * (i + 1)] = blk[HB:2 * HB]
    return full, res


def kernel(**inputs):
    full, _ = run(inputs, trace=False)
    return full
